# revision 7
# baseline (speedup 1.0000x reference)
"""CTAttention Trainium2 kernel — 8 NeuronCores, fully SPMD, no collectives.

Problem: B=2, N=1024, C=512, H=8 heads (hd=64), cluster_num K=8.
reference returns (out, attn_map):
  attn_map[b,c,h,i,j] = (q_i . k_j) * scale  if idx[b,i]==c and idx[b,j]==c else 0
  attn = eps-smoothed softmax of sum_c attn_map  -> out = proj(attn @ v)

Sharding:
  * attn_map planes: core i owns (b = i//4, heads 2*(i%4), 2*(i%4)+1) and
    writes the 16 (c, h_local) planes of that (b, head-pair).  Only the ~1024
    nonzero rows per (b,h) are written (the runner pre-zeroes ExternalOutput
    buffers); rows are produced in cluster-sorted order by a dense matmul,
    column-masked, and scattered with indirect DMA to (c, h, orig_row).
  * out rows: core i owns cluster i for both batches: the host gathers that
    cluster's tokens (both batches concatenated, padded to P each), the device
    computes the eps-smoothed cluster attention + final projection rows, and
    the host scatters rows back by token index.
"""

import math
import os

import numpy as np

LAST_EXEC_NS = None
LAST_RES = None

B, N, C, H, HD, K = 2, 1024, 512, 8, 64, 8
SCALE = HD ** -0.5
EPS = 1e-6
NCORES = 8


def _chunks(P):
    """128-granule chunks of one batch's padded cluster range."""
    out = []
    off = 0
    while off < P:
        sz = min(128, P - off)
        out.append((off, sz))
        off += sz
    return out


def _build(P, has_bias):
    import concourse.bass as bass
    import concourse.mybir as mybir
    import concourse.tile as tile
    from concourse import bacc

    f32 = mybir.dt.float32
    bf16 = mybir.dt.bfloat16
    i32 = mybir.dt.int32
    AF = mybir.ActivationFunctionType
    CH = _chunks(P)
    P2 = 2 * P
    assert P <= 512

    nc = bacc.Bacc("TRN2", target_bir_lowering=False, debug=False,
                   num_devices=NCORES)

    xT_d = nc.declare_dram_parameter("xT", [2, C, N], f32, isOutput=False)
    xsT_d = nc.declare_dram_parameter("xsT", [C, N], f32, isOutput=False)
    msk_d = nc.declare_dram_parameter("msk", [N, N], f32, isOutput=False)
    ridx_d = nc.declare_dram_parameter("ridx", [128, 16], i32, isOutput=False)
    wh_d = nc.declare_dram_parameter("Wh", [C, 256], f32, isOutput=False)
    wq_d = nc.declare_dram_parameter("WqT", [C, C], f32, isOutput=False)
    wk_d = nc.declare_dram_parameter("WkT", [C, C], f32, isOutput=False)
    wv_d = nc.declare_dram_parameter("WvT", [C, C], f32, isOutput=False)
    wp_d = nc.declare_dram_parameter("WpT", [C, C], f32, isOutput=False)
    bp_d = nc.declare_dram_parameter("bproj", [1, C], f32, isOutput=False)
    xg_d = nc.declare_dram_parameter("xgT", [C, P2], f32, isOutput=False)
    pf_d = nc.declare_dram_parameter("pflag", [1, P2], f32, isOutput=False)
    attn_d = nc.declare_dram_parameter("attn_rows", [K * 2 * N, N], f32,
                                       isOutput=True)
    rows_d = nc.declare_dram_parameter("rows_out", [2, P, C], f32,
                                       isOutput=True)

    with tile.TileContext(nc) as tc:
        with (
            tc.tile_pool(name="resident", bufs=1) as rp,
            tc.tile_pool(name="tmp", bufs=2) as tp,
            tc.tile_pool(name="stage", bufs=3) as sp,
            tc.tile_pool(name="scr", bufs=4) as cp,
            tc.tile_pool(name="expp", bufs=8) as ep,
            tc.tile_pool(name="psA", bufs=2, space="PSUM") as psA,
            tc.tile_pool(name="psB", bufs=4, space="PSUM") as psB,
        ):
            # ---------------- batched loads + casts ----------------
            xT = []          # [bb] -> [128, 4, N] f32  (kc in middle dim)
            for bb in range(2):
                t = rp.tile([128, 4, N], f32, name=f"xTf{bb}", tag=f"xTf{bb}")
                nc.sync.dma_start(
                    t[:], xT_d[bb].rearrange("(kc p) n -> p kc n", p=128))
                xT.append(t)
            xsT = rp.tile([128, 4, N], f32, name="xsTf", tag="xsTf")
            nc.sync.dma_start(
                xsT[:], xsT_d[:].rearrange("(kc p) n -> p kc n", p=128))
            msk = []         # [half] -> [128, 4, N] f32 (t = 4*half + mid)
            for half in range(2):
                t = rp.tile([128, 4, N], f32, name=f"msk{half}",
                            tag=f"msk{half}")
                nc.sync.dma_start(
                    t[:], msk_d[4 * 128 * half:4 * 128 * (half + 1),
                                :].rearrange("(tt p) n -> p tt n", p=128))
                msk.append(t)
            ridx = rp.tile([128, 16], i32, name="ridx", tag="ridx")
            nc.sync.dma_start(ridx[:], ridx_d[:])
            wh = rp.tile([128, 4, 256], f32, name="wh", tag="wh")
            nc.sync.dma_start(
                wh[:], wh_d[:].rearrange("(kc p) n -> p kc n", p=128))
            wbf = {}
            for name, dram in (("q", wq_d), ("k", wk_d), ("v", wv_d),
                               ("p", wp_d)):
                ld = tp.tile([128, 4, C], f32, name="wld", tag="wld")
                nc.sync.dma_start(
                    ld[:], dram[:].rearrange("(kc p) n -> p kc n", p=128))
                bt = rp.tile([128, 4, C], bf16, name=f"w{name}b",
                             tag=f"w{name}b")
                nc.vector.tensor_copy(bt[:], ld[:])
                wbf[name] = bt
            xgld = tp.tile([128, 4, P2], f32, name="xgld", tag="xgld")
            nc.sync.dma_start(
                xgld[:], xg_d[:].rearrange("(kc p) n -> p kc n", p=128))
            xg = rp.tile([128, 4, P2], bf16, name="xgb", tag="xgb")
            nc.vector.tensor_copy(xg[:], xgld[:])
            pfld = tp.tile([1, P2], f32, name="pfld", tag="pfld")
            nc.sync.dma_start(pfld[:], pf_d[:])
            pfl = rp.tile([1, P2], bf16, name="pfb", tag="pfb")
            nc.vector.tensor_copy(pfl[:], pfld[:])
            if has_bias:
                bld = tp.tile([1, C], f32, name="bld", tag="bld")
                nc.sync.dma_start(bld[:], bp_d[:])
                bpb = rp.tile([1, C], bf16, name="bpb", tag="bpb")
                nc.vector.tensor_copy(bpb[:], bld[:])
            ones_row = rp.tile([1, 128], bf16, name="ones", tag="ones")
            nc.gpsimd.memset(ones_row[:], 1.0)
            ones_col = rp.tile([128, 1], bf16, name="onesc", tag="onesc")
            nc.gpsimd.memset(ones_col[:], 1.0)
            ident = rp.tile([128, 128], bf16, name="ident", tag="ident")
            nc.gpsimd.memset(ident[:], 0.0)
            nc.gpsimd.affine_select(
                out=ident[:], in_=ident[:],
                compare_op=mybir.AluOpType.not_equal, fill=1.0, base=0,
                pattern=[[-1, 128]], channel_multiplier=1)

            # ---------------- dense attn_map part ----------------
            # q-sorted / k for both local heads in one [128, N] tile each
            qp = psA.tile([128, N], f32, name="qp", tag="bigs")
            for nb in range(2):
                for kc in range(4):
                    nc.tensor.matmul(
                        qp[:, 512 * nb:512 * (nb + 1)],
                        wh[:, kc, 0:128],
                        xsT[:, kc, 512 * nb:512 * (nb + 1)],
                        start=(kc == 0), stop=(kc == 3))
            qsT = rp.tile([128, N], f32, name="qsT", tag="qsT")
            nc.scalar.mul(qsT[:], qp[:], SCALE)  # fold attention scale
            kp = psA.tile([128, N], f32, name="kp", tag="bigs")
            for nb in range(2):
                for kc in range(4):
                    nc.tensor.matmul(
                        kp[:, 512 * nb:512 * (nb + 1)],
                        wh[:, kc, 128:256],
                        xT[0][:, kc, 512 * nb:512 * (nb + 1)],
                        start=(kc == 0), stop=(kc == 3))
            kT = rp.tile([128, N], f32, name="kTt", tag="kTt")
            nc.scalar.copy(kT[:], kp[:])
            # sorted-row score tiles -> mask -> scatter to (c, h, orig) rows
            for hh in range(2):
                for t in range(8):
                    sps = psA.tile([128, N], f32, name="sps", tag="bigs")
                    for nb in range(2):
                        nc.tensor.matmul(
                            sps[:, 512 * nb:512 * (nb + 1)],
                            qsT[64 * hh:64 * (hh + 1), 128 * t:128 * (t + 1)],
                            kT[64 * hh:64 * (hh + 1),
                               512 * nb:512 * (nb + 1)],
                            start=True, stop=True)
                    masked = sp.tile([128, N], f32, name="masked",
                                     tag="masked")
                    nc.vector.tensor_tensor(masked[:], sps[:],
                                            msk[t // 4][:, t % 4, :],
                                            op=mybir.AluOpType.mult)
                    nc.gpsimd.indirect_dma_start(
                        out=attn_d[:],
                        out_offset=bass.IndirectOffsetOnAxis(
                            ap=ridx[:, 8 * hh + t:8 * hh + t + 1], axis=0),
                        in_=masked[:],
                        in_offset=None)

            # ---------------- cluster (out rows) part ----------------
            # xsum -> Vsum per batch (scaled by eps/N)
            vs = []
            for bb in range(2):
                xsum = [None] * 4
                for kc in range(4):
                    red = cp.tile([128, 1], f32, name="xsum", tag="xsum")
                    nc.vector.reduce_sum(red[:], xT[bb][:, kc, :],
                                         axis=mybir.AxisListType.X)
                    xb = cp.tile([128, 1], bf16, name="xsumb", tag="xsumb")
                    nc.vector.tensor_copy(xb[:], red[:])
                    xsum[kc] = xb
                vp = psB.tile([1, C], f32, name="vp", tag="psc")
                for kc in range(4):
                    nc.tensor.matmul(vp[:], xsum[kc][:], wbf["v"][:, kc, :],
                                     start=(kc == 0), stop=(kc == 3))
                vt = rp.tile([1, C], bf16, name=f"vsum{bb}", tag=f"vsum{bb}")
                nc.scalar.mul(vt[:], vp[:], EPS / N)
                vs.append(vt)

            # gathered q/k per head over the 2P concat (+ ext row),
            # v per (bb, chunk, head-pair)
            qg = [None] * H
            kg = [None] * H
            vg = [[[None] * 4 for _ in CH] for _ in range(2)]
            for hp in range(4):
                qp2 = psB.tile([128, P2], f32, name="qp2", tag="psc")
                for kc in range(4):
                    nc.tensor.matmul(
                        qp2[:], wbf["q"][:, kc, 128 * hp:128 * (hp + 1)],
                        xg[:, kc, :], start=(kc == 0), stop=(kc == 3))
                kp2 = psB.tile([128, P2], f32, name="kp2", tag="psc")
                for kc in range(4):
                    nc.tensor.matmul(
                        kp2[:], wbf["k"][:, kc, 128 * hp:128 * (hp + 1)],
                        xg[:, kc, :], start=(kc == 0), stop=(kc == 3))
                for hh in range(2):
                    h = 2 * hp + hh
                    qe = rp.tile([65, P2], bf16, name=f"qg{h}", tag=f"qg{h}")
                    nc.scalar.mul(qe[0:64, :],
                                  qp2[64 * hh:64 * (hh + 1), :], SCALE)
                    nc.gpsimd.memset(qe[64:65, :], 1.0)
                    qg[h] = qe
                    ke = rp.tile([65, P2], bf16, name=f"kg{h}", tag=f"kg{h}")
                    nc.scalar.copy(ke[0:64, :],
                                   kp2[64 * hh:64 * (hh + 1), :])
                    nc.vector.tensor_copy(ke[64:65, :], pfl[:])
                    kg[h] = ke
                for bb in range(2):
                    for ci, (off, sz) in enumerate(CH):
                        vp2 = psB.tile([128, 128], f32, name="vp2", tag="psc")
                        for kc in range(4):
                            nc.tensor.matmul(
                                vp2[0:sz, :],
                                xg[:, kc, bb * P + off:bb * P + off + sz],
                                wbf["v"][:, kc, 128 * hp:128 * (hp + 1)],
                                start=(kc == 0), stop=(kc == 3))
                        vt2 = rp.tile([128, 128], bf16, name=f"vg{bb}{ci}{hp}",
                                      tag=f"vg{bb}{ci}{hp}")
                        nc.scalar.copy(vt2[0:sz, :], vp2[0:sz, :])
                        vg[bb][ci][hp] = vt2

            # per (batch, head): eps-smoothed cluster attention rows
            obf = [[rp.tile([128, C], bf16, name=f"obf{bb}{ci}",
                            tag=f"obf{bb}{ci}") for ci in range(len(CH))]
                   for bb in range(2)]
            for bb in range(2):
                for h in range(H):
                    hp, hh = divmod(h, 2)
                    # key-major exp tiles
                    expT = []
                    for (joff, jsz) in CH:
                        spT = psB.tile([128, P], f32, name="spT", tag="psc")
                        nc.tensor.matmul(
                            spT[0:jsz, :],
                            kg[h][:, bb * P + joff:bb * P + joff + jsz],
                            qg[h][:, bb * P:bb * P + P],
                            start=True, stop=True)
                        et = ep.tile([128, P], bf16, name="expT", tag="expT")
                        nc.scalar.activation(et[0:jsz, :], spT[0:jsz, :],
                                             AF.Exp)
                        expT.append(et)
                    # Z (column sums of expT) per i-chunk, then 1/(Z+eps)
                    recips = []
                    for ci, (ioff, isz) in enumerate(CH):
                        zp = psB.tile([128, 1], f32, name="zp", tag="psc")
                        for ji, (joff, jsz) in enumerate(CH):
                            nc.tensor.matmul(
                                zp[0:isz, :],
                                expT[ji][0:jsz, ioff:ioff + isz],
                                ones_col[0:jsz, :],
                                start=(ji == 0), stop=(ji == len(CH) - 1))
                        ze = cp.tile([128, 1], f32, name="ze", tag="ze")
                        nc.vector.tensor_scalar_add(ze[0:isz, :],
                                                    zp[0:isz, :], EPS)
                        rc = cp.tile([128, 1], f32, name="rc", tag="rc",
                                     bufs=6)
                        nc.vector.reciprocal(rc[0:isz, :], ze[0:isz, :])
                        recips.append(rc)
                    # num = exp @ v + (eps/N) * Vsum ; rows scaled by recip
                    for ci, (ioff, isz) in enumerate(CH):
                        np_ = psB.tile([128, 64], f32, name="nump", tag="psc")
                        for ji, (joff, jsz) in enumerate(CH):
                            nc.tensor.matmul(
                                np_[0:isz, :],
                                expT[ji][0:jsz, ioff:ioff + isz],
                                vg[bb][ji][hp][0:jsz, 64 * hh:64 * (hh + 1)],
                                start=(ji == 0), stop=False)
                        nc.tensor.matmul(np_[0:isz, :], ones_row[:, 0:isz],
                                         vs[bb][:, 64 * h:64 * (h + 1)],
                                         start=False, stop=True)
                        nc.vector.tensor_scalar_mul(
                            obf[bb][ci][0:isz, 64 * h:64 * (h + 1)],
                            np_[0:isz, :], recips[ci][0:isz, :])

            # transpose o, project, store out rows
            for bb in range(2):
                oT = [[None] * len(CH) for _ in range(4)]
                for ci, (ioff, isz) in enumerate(CH):
                    for cc in range(4):
                        tps = psB.tile([128, 128], bf16, name="psc_t",
                                       tag="psc")
                        nc.tensor.transpose(
                            tps[:, 0:isz],
                            obf[bb][ci][0:isz, 128 * cc:128 * (cc + 1)],
                            ident[0:isz, 0:isz])
                        ot = cp.tile([128, 128], bf16, name=f"oT{bb}{cc}{ci}",
                                     tag=f"oT{cc}{ci}")
                        nc.scalar.copy(ot[:, 0:isz], tps[:, 0:isz])
                        oT[cc][ci] = ot
                for ci, (ioff, isz) in enumerate(CH):
                    fp = psB.tile([128, C], f32, name="fp", tag="psc")
                    for cc in range(4):
                        nc.tensor.matmul(fp[0:isz, :], oT[cc][ci][:, 0:isz],
                                         wbf["p"][:, cc, :],
                                         start=(cc == 0),
                                         stop=(cc == 3 and not has_bias))
                    if has_bias:
                        nc.tensor.matmul(fp[0:isz, :], ones_row[:, 0:isz],
                                         bpb[:], start=False, stop=True)
                    fs = sp.tile([128, C], f32, name="fs", tag="fs")
                    nc.scalar.copy(fs[0:isz, :], fp[0:isz, :])
                    nc.sync.dma_start(rows_d[bb, ioff:ioff + isz, :],
                                      fs[0:isz, :])

    nc.compile()
    return nc


def kernel(**inputs):
    from concourse.bass_utils import run_bass_kernel_spmd

    x = np.asarray(inputs["x_token"], np.float32)             # (B, N, C)
    idx = np.asarray(inputs["idx_cluster"]).astype(np.int64)  # (B, N)
    Wq = np.asarray(inputs["Wq"], np.float32)
    Wk = np.asarray(inputs["Wk"], np.float32)
    Wv = np.asarray(inputs["Wv"], np.float32)
    Wproj = np.asarray(inputs["Wproj"], np.float32)
    bproj = np.asarray(inputs["bproj"], np.float32)
    assert x.shape == (B, N, C) and idx.shape == (B, N)
    assert int(np.asarray(inputs["cluster_num"])) == K

    # ---- host-side index/shard prep
    perm = [np.argsort(idx[b], kind="stable") for b in range(B)]
    sortc = [idx[b][perm[b]] for b in range(B)]
    ids = [[np.where(idx[b] == c)[0] for c in range(K)] for b in range(B)]
    maxsz = max(len(ids[b][c]) for b in range(B) for c in range(K))
    P = max(32, 32 * math.ceil(maxsz / 32))

    xT = [np.ascontiguousarray(x[b].T) for b in range(B)]
    xsT = [np.ascontiguousarray(x[b][perm[b]].T) for b in range(B)]
    msk = [(sortc[b][:, None] == idx[b][None, :]).astype(np.float32)
           for b in range(B)]
    WqT = np.ascontiguousarray(Wq.T)
    WkT = np.ascontiguousarray(Wk.T)
    WvT = np.ascontiguousarray(Wv.T)
    WpT = np.ascontiguousarray(Wproj.T)
    has_bias = bool(np.any(bproj != 0))

    in_maps = []
    for core in range(NCORES):
        b = core // 4
        h0 = 2 * (core % 4)
        c = core
        # scatter row ids: shard row = (cluster*2 + h_local)*N + orig_row
        ridx = np.zeros((128, 16), np.int32)
        for hl in range(2):
            rows = (sortc[b] * 2 + hl) * N + perm[b]
            for t in range(8):
                ridx[:, 8 * hl + t] = rows[128 * t:128 * (t + 1)]
        # gathered cluster tokens, [own batch | other batch] concat
        xgT = np.zeros((C, 2 * P), np.float32)
        pfl = np.zeros((1, 2 * P), np.float32)
        for slot, bb in enumerate((b, 1 - b)):
            tok = ids[bb][c]
            xgT[:, slot * P:slot * P + len(tok)] = x[bb][tok].T
            pfl[0, slot * P + len(tok):(slot + 1) * P] = -1e9
        in_maps.append({
            "xT": np.ascontiguousarray(np.stack([xT[b], xT[1 - b]])),
            "xsT": xsT[b],
            "msk": msk[b],
            "ridx": ridx,
            "Wh": np.ascontiguousarray(np.concatenate(
                [WqT[:, 64 * h0:64 * (h0 + 2)],
                 WkT[:, 64 * h0:64 * (h0 + 2)]], axis=1)),
            "WqT": WqT, "WkT": WkT, "WvT": WvT, "WpT": WpT,
            "bproj": np.ascontiguousarray(bproj.reshape(1, C)),
            "xgT": xgT,
            "pflag": pfl,
        })

    nc = _build(P, has_bias)
    trace = bool(os.environ.get("CTA_TRACE"))
    res = run_bass_kernel_spmd(nc, in_maps, core_ids=list(range(NCORES)),
                               trace=trace)
    global LAST_EXEC_NS, LAST_RES
    LAST_EXEC_NS = res.exec_time_ns
    LAST_RES = res

    # ---- unshard
    attn_map = np.empty((B, K, H, N, N), np.float32)
    out = np.empty((B, N, C), np.float32)
    for core in range(NCORES):
        b = core // 4
        h0 = 2 * (core % 4)
        c = core
        shard = res.results[core]["attn_rows"].reshape(K, 2, N, N)
        attn_map[b, :, h0:h0 + 2] = shard
        rows = res.results[core]["rows_out"]          # (2, P, C)
        for slot, bb in enumerate((b, 1 - b)):
            tok = ids[bb][c]
            out[bb, tok] = rows[slot, :len(tok)]
    return out, attn_map


# revision 9
# speedup vs baseline: 1.1266x; 1.1266x over previous
"""CTAttention Trainium2 kernel — 8 NeuronCores, fully SPMD, no collectives.

Problem: B=2, N=1024, C=512, H=8 heads (hd=64), cluster_num K=8.
reference returns (out, attn_map):
  attn_map[b,c,h,i,j] = (q_i . k_j) * scale  if idx[b,i]==c and idx[b,j]==c else 0
  attn = eps-smoothed softmax of sum_c attn_map  -> out = proj(attn @ v)

Sharding:
  * attn_map planes: core i owns (b = i//4, heads 2*(i%4), 2*(i%4)+1) and
    writes the 16 (c, h_local) planes of that (b, head-pair).  Only the ~1024
    nonzero rows per (b,h) are written (the runner pre-zeroes ExternalOutput
    buffers); rows are produced in cluster-sorted order by a dense matmul,
    column-masked, and scattered with indirect DMA to (c, h, orig_row).
  * out rows: core i owns cluster i for both batches: the host gathers that
    cluster's tokens (both batches concatenated, padded to P each), the device
    computes the eps-smoothed cluster attention + final projection rows, and
    the host scatters rows back by token index.
"""

import math
import os

import numpy as np

LAST_EXEC_NS = None
LAST_RES = None

B, N, C, H, HD, K = 2, 1024, 512, 8, 64, 8
SCALE = HD ** -0.5
EPS = 1e-6
NCORES = 8


def _chunks(P):
    """128-granule chunks of one batch's padded cluster range."""
    out = []
    off = 0
    while off < P:
        sz = min(128, P - off)
        out.append((off, sz))
        off += sz
    return out


def _build(P, has_bias):
    import concourse.bass as bass
    import concourse.mybir as mybir
    import concourse.tile as tile
    from concourse import bacc

    f32 = mybir.dt.float32
    bf16 = mybir.dt.bfloat16
    i32 = mybir.dt.int32
    AF = mybir.ActivationFunctionType
    CH = _chunks(P)
    P2 = 2 * P
    assert P <= 512

    nc = bacc.Bacc("TRN2", target_bir_lowering=False, debug=False,
                   num_devices=NCORES)

    xT_d = nc.declare_dram_parameter("xT", [2, C, N], f32, isOutput=False)
    xsT_d = nc.declare_dram_parameter("xsT", [C, N], f32, isOutput=False)
    msk_d = nc.declare_dram_parameter("msk", [N, N], f32, isOutput=False)
    ridx_d = nc.declare_dram_parameter("ridx", [128, 16], i32, isOutput=False)
    wh_d = nc.declare_dram_parameter("Wh", [C, 256], f32, isOutput=False)
    wq_d = nc.declare_dram_parameter("WqT", [C, C], f32, isOutput=False)
    wk_d = nc.declare_dram_parameter("WkT", [C, C], f32, isOutput=False)
    wv_d = nc.declare_dram_parameter("WvT", [C, C], f32, isOutput=False)
    wp_d = nc.declare_dram_parameter("WpT", [C, C], f32, isOutput=False)
    bp_d = nc.declare_dram_parameter("bproj", [1, C], f32, isOutput=False)
    xg_d = nc.declare_dram_parameter("xgT", [C, P2], f32, isOutput=False)
    pf_d = nc.declare_dram_parameter("pflag", [1, P2], f32, isOutput=False)
    attn_d = nc.declare_dram_parameter("attn_rows", [K * 2 * N, N], f32,
                                       isOutput=True)
    rows_d = nc.declare_dram_parameter("rows_out", [2, P, C], f32,
                                       isOutput=True)

    with tile.TileContext(nc) as tc:
        with (
            tc.tile_pool(name="resident", bufs=1) as rp,
            tc.tile_pool(name="tmp", bufs=2) as tp,
            tc.tile_pool(name="stage", bufs=3) as sp,
            tc.tile_pool(name="scr", bufs=4) as cp,
            tc.tile_pool(name="expp", bufs=8) as ep,
            tc.tile_pool(name="psA", bufs=2, space="PSUM") as psA,
            tc.tile_pool(name="psB", bufs=4, space="PSUM") as psB,
        ):
            # ---------------- batched loads + casts ----------------
            # order: cluster-part inputs first (small, unblock PE quickly),
            # then dense inputs, masks last (needed latest)
            wbf = {}
            for name, dram in (("v", wv_d), ("q", wq_d), ("k", wk_d),
                               ("p", wp_d)):
                ld = tp.tile([128, 4, C], f32, name="wld", tag="wld")
                nc.sync.dma_start(
                    ld[:], dram[:].rearrange("(kc p) n -> p kc n", p=128))
                bt = rp.tile([128, 4, C], bf16, name=f"w{name}b",
                             tag=f"w{name}b")
                if name in ("v", "q"):
                    nc.scalar.copy(bt[:], ld[:])
                else:
                    nc.vector.tensor_copy(bt[:], ld[:])
                wbf[name] = bt
            xgld = tp.tile([128, 4, P2], f32, name="xgld", tag="xgld")
            nc.sync.dma_start(
                xgld[:], xg_d[:].rearrange("(kc p) n -> p kc n", p=128))
            xg = rp.tile([128, 4, P2], bf16, name="xgb", tag="xgb")
            nc.vector.tensor_copy(xg[:], xgld[:])
            pfld = tp.tile([1, P2], f32, name="pfld", tag="pfld")
            nc.sync.dma_start(pfld[:], pf_d[:])
            pfl = rp.tile([1, P2], bf16, name="pfb", tag="pfb")
            nc.vector.tensor_copy(pfl[:], pfld[:])
            wh = rp.tile([128, 4, 256], f32, name="wh", tag="wh")
            nc.sync.dma_start(
                wh[:], wh_d[:].rearrange("(kc p) n -> p kc n", p=128))
            xsT = rp.tile([128, 4, N], f32, name="xsTf", tag="xsTf")
            nc.sync.dma_start(
                xsT[:], xsT_d[:].rearrange("(kc p) n -> p kc n", p=128))
            xT = []          # [bb] -> [128, 4, N] f32  (kc in middle dim)
            for bb in range(2):
                t = rp.tile([128, 4, N], f32, name=f"xTf{bb}", tag=f"xTf{bb}")
                nc.sync.dma_start(
                    t[:], xT_d[bb].rearrange("(kc p) n -> p kc n", p=128))
                xT.append(t)
            ridx = rp.tile([128, 16], i32, name="ridx", tag="ridx")
            nc.sync.dma_start(ridx[:], ridx_d[:])
            msk = []         # [half] -> [128, 4, N] f32 (t = 4*half + mid)
            for half in range(2):
                t = rp.tile([128, 4, N], f32, name=f"msk{half}",
                            tag=f"msk{half}")
                nc.sync.dma_start(
                    t[:], msk_d[4 * 128 * half:4 * 128 * (half + 1),
                                :].rearrange("(tt p) n -> p tt n", p=128))
                msk.append(t)
            if has_bias:
                bld = tp.tile([1, C], f32, name="bld", tag="bld")
                nc.sync.dma_start(bld[:], bp_d[:])
                bpb = rp.tile([1, C], bf16, name="bpb", tag="bpb")
                nc.vector.tensor_copy(bpb[:], bld[:])
            ones_row = rp.tile([1, 128], bf16, name="ones", tag="ones")
            nc.gpsimd.memset(ones_row[:], 1.0)
            ident = rp.tile([128, 128], bf16, name="ident", tag="ident")
            nc.gpsimd.memset(ident[:], 0.0)
            nc.gpsimd.affine_select(
                out=ident[:], in_=ident[:],
                compare_op=mybir.AluOpType.not_equal, fill=1.0, base=0,
                pattern=[[-1, 128]], channel_multiplier=1)

            # ---------------- cluster part: projections first ----------------
            # (small inputs -> PE starts early and HAM-warms during big loads)
            # gathered q/k per head over the 2P concat (+ ext row)
            qg = [None] * H
            kg = [None] * H
            for hp in range(4):
                qp2 = psB.tile([128, P2], f32, name="qp2", tag="psc")
                for kc in range(4):
                    nc.tensor.matmul(
                        qp2[:], wbf["q"][:, kc, 128 * hp:128 * (hp + 1)],
                        xg[:, kc, :], start=(kc == 0), stop=(kc == 3))
                kp2 = psB.tile([128, P2], f32, name="kp2", tag="psc")
                for kc in range(4):
                    nc.tensor.matmul(
                        kp2[:], wbf["k"][:, kc, 128 * hp:128 * (hp + 1)],
                        xg[:, kc, :], start=(kc == 0), stop=(kc == 3))
                for hh in range(2):
                    h = 2 * hp + hh
                    qe = rp.tile([65, P2], bf16, name=f"qg{h}", tag=f"qg{h}")
                    nc.scalar.mul(qe[0:64, :],
                                  qp2[64 * hh:64 * (hh + 1), :], SCALE)
                    nc.gpsimd.memset(qe[64:65, :], 1.0)
                    qg[h] = qe
                    ke = rp.tile([65, P2], bf16, name=f"kg{h}", tag=f"kg{h}")
                    nc.scalar.copy(ke[0:64, :],
                                   kp2[64 * hh:64 * (hh + 1), :])
                    nc.vector.tensor_copy(ke[64:65, :], pfl[:])
                    kg[h] = ke
            # v for all heads at once; per head-pair tiles hold
            # [v_h0 (64) | ones | v_h1 (64) | ones] so the num matmul also
            # produces the softmax denominator Z in its last column.
            vg = [[[None] * 4 for _ in CH] for _ in range(2)]
            for bb in range(2):
                for ci, (off, sz) in enumerate(CH):
                    vp2 = psB.tile([128, C], f32, name="vp2", tag="psc")
                    for kc in range(4):
                        nc.tensor.matmul(
                            vp2[0:sz, :],
                            xg[:, kc, bb * P + off:bb * P + off + sz],
                            wbf["v"][:, kc, :],
                            start=(kc == 0), stop=(kc == 3))
                    for hp in range(4):
                        vt2 = rp.tile([128, 130], bf16,
                                      name=f"vg{bb}{ci}{hp}",
                                      tag=f"vg{bb}{ci}{hp}")
                        nc.scalar.copy(vt2[0:sz, 0:64],
                                       vp2[0:sz, 128 * hp:128 * hp + 64])
                        nc.scalar.copy(vt2[0:sz, 65:129],
                                       vp2[0:sz, 128 * hp + 64:128 * hp + 128])
                        nc.gpsimd.memset(vt2[:, 64:65], 1.0)
                        nc.gpsimd.memset(vt2[:, 129:130], 1.0)
                        vg[bb][ci][hp] = vt2
            # xsum -> Vsum per batch (scaled by eps/N)
            vs = []
            for bb in range(2):
                xsum = [None] * 4
                for kc in range(4):
                    red = cp.tile([128, 1], f32, name="xsum", tag="xsum")
                    nc.vector.reduce_sum(red[:], xT[bb][:, kc, :],
                                         axis=mybir.AxisListType.X)
                    xb = cp.tile([128, 1], bf16, name="xsumb", tag="xsumb")
                    nc.vector.tensor_copy(xb[:], red[:])
                    xsum[kc] = xb
                vp = psB.tile([1, C], f32, name="vp", tag="psc")
                for kc in range(4):
                    nc.tensor.matmul(vp[:], xsum[kc][:], wbf["v"][:, kc, :],
                                     start=(kc == 0), stop=(kc == 3))
                vt = rp.tile([1, C], bf16, name=f"vsum{bb}", tag=f"vsum{bb}")
                nc.scalar.mul(vt[:], vp[:], EPS / N)
                vs.append(vt)

            # ---------------- dense attn_map part ----------------
            # q-sorted / k for both local heads in one [128, N] tile each
            qp = psA.tile([128, N], f32, name="qp", tag="bigs")
            for nb in range(2):
                for kc in range(4):
                    nc.tensor.matmul(
                        qp[:, 512 * nb:512 * (nb + 1)],
                        wh[:, kc, 0:128],
                        xsT[:, kc, 512 * nb:512 * (nb + 1)],
                        start=(kc == 0), stop=(kc == 3))
            qsT = rp.tile([128, N], f32, name="qsT", tag="qsT")
            nc.scalar.mul(qsT[:], qp[:], SCALE)  # fold attention scale
            kp = psA.tile([128, N], f32, name="kp", tag="bigs")
            for nb in range(2):
                for kc in range(4):
                    nc.tensor.matmul(
                        kp[:, 512 * nb:512 * (nb + 1)],
                        wh[:, kc, 128:256],
                        xT[0][:, kc, 512 * nb:512 * (nb + 1)],
                        start=(kc == 0), stop=(kc == 3))
            kT = rp.tile([128, N], f32, name="kTt", tag="kTt")
            nc.scalar.copy(kT[:], kp[:])
            # sorted-row score tiles -> mask -> scatter to (c, h, orig) rows
            for hh in range(2):
                for t in range(8):
                    sps = psA.tile([128, N], f32, name="sps", tag="bigs")
                    for nb in range(2):
                        nc.tensor.matmul(
                            sps[:, 512 * nb:512 * (nb + 1)],
                            qsT[64 * hh:64 * (hh + 1), 128 * t:128 * (t + 1)],
                            kT[64 * hh:64 * (hh + 1),
                               512 * nb:512 * (nb + 1)],
                            start=True, stop=True)
                    masked = sp.tile([128, N], f32, name="masked",
                                     tag="masked")
                    nc.vector.tensor_tensor(masked[:], sps[:],
                                            msk[t // 4][:, t % 4, :],
                                            op=mybir.AluOpType.mult)
                    nc.gpsimd.indirect_dma_start(
                        out=attn_d[:],
                        out_offset=bass.IndirectOffsetOnAxis(
                            ap=ridx[:, 8 * hh + t:8 * hh + t + 1], axis=0),
                        in_=masked[:],
                        in_offset=None)

            # ------------- cluster attention (eps-smoothed) -------------
            obf = [[rp.tile([128, C], bf16, name=f"obf{bb}{ci}",
                            tag=f"obf{bb}{ci}") for ci in range(len(CH))]
                   for bb in range(2)]
            for bb in range(2):
                for h in range(H):
                    hp, hh = divmod(h, 2)
                    # key-major exp tiles
                    expT = []
                    for (joff, jsz) in CH:
                        spT = psB.tile([128, P], f32, name="spT", tag="psc")
                        nc.tensor.matmul(
                            spT[0:jsz, :],
                            kg[h][:, bb * P + joff:bb * P + joff + jsz],
                            qg[h][:, bb * P:bb * P + P],
                            start=True, stop=True)
                        et = ep.tile([128, P], bf16, name="expT", tag="expT")
                        nc.scalar.activation(et[0:jsz, :], spT[0:jsz, :],
                                             AF.Exp)
                        expT.append(et)
                    # num[:, 0:64] = exp @ v + (eps/N) * Vsum,
                    # num[:, 64] = Z; rows scaled by 1/(Z+eps)
                    for ci, (ioff, isz) in enumerate(CH):
                        np_ = psB.tile([128, 65], f32, name="nump", tag="psc")
                        for ji, (joff, jsz) in enumerate(CH):
                            nc.tensor.matmul(
                                np_[0:isz, :],
                                expT[ji][0:jsz, ioff:ioff + isz],
                                vg[bb][ji][hp][0:jsz,
                                               65 * hh:65 * hh + 65],
                                start=(ji == 0), stop=False)
                        nc.tensor.matmul(np_[0:isz, 0:64],
                                         ones_row[:, 0:isz],
                                         vs[bb][:, 64 * h:64 * (h + 1)],
                                         start=False, stop=True,
                                         skip_group_check=True)
                        ze = cp.tile([128, 1], f32, name="ze", tag="ze")
                        nc.vector.tensor_scalar_add(ze[0:isz, :],
                                                    np_[0:isz, 64:65], EPS)
                        rc = cp.tile([128, 1], f32, name="rc", tag="rc",
                                     bufs=6)
                        nc.vector.reciprocal(rc[0:isz, :], ze[0:isz, :])
                        nc.vector.tensor_scalar_mul(
                            obf[bb][ci][0:isz, 64 * h:64 * (h + 1)],
                            np_[0:isz, 0:64], rc[0:isz, :])

            # transpose o, project, store out rows
            for bb in range(2):
                oT = [[None] * len(CH) for _ in range(4)]
                for ci, (ioff, isz) in enumerate(CH):
                    for cc in range(4):
                        tps = psB.tile([128, 128], bf16, name="psc_t",
                                       tag="psc")
                        nc.tensor.transpose(
                            tps[:, 0:isz],
                            obf[bb][ci][0:isz, 128 * cc:128 * (cc + 1)],
                            ident[0:isz, 0:isz])
                        ot = cp.tile([128, 128], bf16, name=f"oT{bb}{cc}{ci}",
                                     tag=f"oT{cc}{ci}")
                        nc.scalar.copy(ot[:, 0:isz], tps[:, 0:isz])
                        oT[cc][ci] = ot
                for ci, (ioff, isz) in enumerate(CH):
                    fp = psB.tile([128, C], f32, name="fp", tag="psc")
                    for cc in range(4):
                        nc.tensor.matmul(fp[0:isz, :], oT[cc][ci][:, 0:isz],
                                         wbf["p"][:, cc, :],
                                         start=(cc == 0),
                                         stop=(cc == 3 and not has_bias))
                    if has_bias:
                        nc.tensor.matmul(fp[0:isz, :], ones_row[:, 0:isz],
                                         bpb[:], start=False, stop=True)
                    fs = sp.tile([128, C], f32, name="fs", tag="fs")
                    nc.scalar.copy(fs[0:isz, :], fp[0:isz, :])
                    nc.sync.dma_start(rows_d[bb, ioff:ioff + isz, :],
                                      fs[0:isz, :])

    nc.compile()
    return nc


def kernel(**inputs):
    from concourse.bass_utils import run_bass_kernel_spmd

    x = np.asarray(inputs["x_token"], np.float32)             # (B, N, C)
    idx = np.asarray(inputs["idx_cluster"]).astype(np.int64)  # (B, N)
    Wq = np.asarray(inputs["Wq"], np.float32)
    Wk = np.asarray(inputs["Wk"], np.float32)
    Wv = np.asarray(inputs["Wv"], np.float32)
    Wproj = np.asarray(inputs["Wproj"], np.float32)
    bproj = np.asarray(inputs["bproj"], np.float32)
    assert x.shape == (B, N, C) and idx.shape == (B, N)
    assert int(np.asarray(inputs["cluster_num"])) == K

    # ---- host-side index/shard prep
    perm = [np.argsort(idx[b], kind="stable") for b in range(B)]
    sortc = [idx[b][perm[b]] for b in range(B)]
    ids = [[np.where(idx[b] == c)[0] for c in range(K)] for b in range(B)]
    maxsz = max(len(ids[b][c]) for b in range(B) for c in range(K))
    P = max(32, 32 * math.ceil(maxsz / 32))

    xT = [np.ascontiguousarray(x[b].T) for b in range(B)]
    xsT = [np.ascontiguousarray(x[b][perm[b]].T) for b in range(B)]
    msk = [(sortc[b][:, None] == idx[b][None, :]).astype(np.float32)
           for b in range(B)]
    WqT = np.ascontiguousarray(Wq.T)
    WkT = np.ascontiguousarray(Wk.T)
    WvT = np.ascontiguousarray(Wv.T)
    WpT = np.ascontiguousarray(Wproj.T)
    has_bias = bool(np.any(bproj != 0))

    in_maps = []
    for core in range(NCORES):
        b = core // 4
        h0 = 2 * (core % 4)
        c = core
        # scatter row ids: shard row = (cluster*2 + h_local)*N + orig_row
        ridx = np.zeros((128, 16), np.int32)
        for hl in range(2):
            rows = (sortc[b] * 2 + hl) * N + perm[b]
            for t in range(8):
                ridx[:, 8 * hl + t] = rows[128 * t:128 * (t + 1)]
        # gathered cluster tokens, [own batch | other batch] concat
        xgT = np.zeros((C, 2 * P), np.float32)
        pfl = np.zeros((1, 2 * P), np.float32)
        for slot, bb in enumerate((b, 1 - b)):
            tok = ids[bb][c]
            xgT[:, slot * P:slot * P + len(tok)] = x[bb][tok].T
            pfl[0, slot * P + len(tok):(slot + 1) * P] = -1e9
        in_maps.append({
            "xT": np.ascontiguousarray(np.stack([xT[b], xT[1 - b]])),
            "xsT": xsT[b],
            "msk": msk[b],
            "ridx": ridx,
            "Wh": np.ascontiguousarray(np.concatenate(
                [WqT[:, 64 * h0:64 * (h0 + 2)],
                 WkT[:, 64 * h0:64 * (h0 + 2)]], axis=1)),
            "WqT": WqT, "WkT": WkT, "WvT": WvT, "WpT": WpT,
            "bproj": np.ascontiguousarray(bproj.reshape(1, C)),
            "xgT": xgT,
            "pflag": pfl,
        })

    nc = _build(P, has_bias)
    trace = bool(os.environ.get("CTA_TRACE"))
    res = run_bass_kernel_spmd(nc, in_maps, core_ids=list(range(NCORES)),
                               trace=trace)
    global LAST_EXEC_NS, LAST_RES
    LAST_EXEC_NS = res.exec_time_ns
    LAST_RES = res

    # ---- unshard
    attn_map = np.empty((B, K, H, N, N), np.float32)
    out = np.empty((B, N, C), np.float32)
    for core in range(NCORES):
        b = core // 4
        h0 = 2 * (core % 4)
        c = core
        shard = res.results[core]["attn_rows"].reshape(K, 2, N, N)
        attn_map[b, :, h0:h0 + 2] = shard
        rows = res.results[core]["rows_out"]          # (2, P, C)
        for slot, bb in enumerate((b, 1 - b)):
            tok = ids[bb][c]
            out[bb, tok] = rows[slot, :len(tok)]
    return out, attn_map


# revision 11
# speedup vs baseline: 1.2573x; 1.1160x over previous
"""CTAttention Trainium2 kernel — 8 NeuronCores, fully SPMD, no collectives.

Problem: B=2, N=1024, C=512, H=8 heads (hd=64), cluster_num K=8.
reference returns (out, attn_map):
  attn_map[b,c,h,i,j] = (q_i . k_j) * scale  if idx[b,i]==c and idx[b,j]==c else 0
  attn = eps-smoothed softmax of sum_c attn_map  -> out = proj(attn @ v)

Sharding:
  * attn_map planes: core i owns (b = i//4, heads 2*(i%4), 2*(i%4)+1) and
    writes the 16 (c, h_local) planes of that (b, head-pair).  Only the ~1024
    nonzero rows per (b,h) are written (the runner pre-zeroes ExternalOutput
    buffers); rows are produced in cluster-sorted order by a dense matmul,
    column-masked, and scattered with indirect DMA to (c, h, orig_row).
  * out rows: core i owns cluster i for both batches: the host gathers that
    cluster's tokens (both batches concatenated, padded to P each), the device
    computes the eps-smoothed cluster attention + final projection rows, and
    the host scatters rows back by token index.
"""

import math
import os

import numpy as np

LAST_EXEC_NS = None
LAST_RES = None

B, N, C, H, HD, K = 2, 1024, 512, 8, 64, 8
SCALE = HD ** -0.5
EPS = 1e-6
NCORES = 8


def _chunks(P):
    """128-granule chunks of one batch's padded cluster range."""
    out = []
    off = 0
    while off < P:
        sz = min(128, P - off)
        out.append((off, sz))
        off += sz
    return out


def _build(P, has_bias):
    import concourse.bass as bass
    import concourse.mybir as mybir
    import concourse.tile as tile
    from concourse import bacc

    f32 = mybir.dt.float32
    bf16 = mybir.dt.bfloat16
    i32 = mybir.dt.int32
    AF = mybir.ActivationFunctionType
    CH = _chunks(P)
    P2 = 2 * P
    assert P <= 512

    nc = bacc.Bacc("TRN2", target_bir_lowering=False, debug=False,
                   num_devices=NCORES)

    xT_d = nc.declare_dram_parameter("xT", [2, C, N], f32, isOutput=False)
    xsT_d = nc.declare_dram_parameter("xsT", [C, N], f32, isOutput=False)
    msk_d = nc.declare_dram_parameter("msk", [N, N], f32, isOutput=False)
    ridx_d = nc.declare_dram_parameter("ridx", [128, 16], i32, isOutput=False)
    wh_d = nc.declare_dram_parameter("Wh", [C, 256], f32, isOutput=False)
    wq_d = nc.declare_dram_parameter("WqT", [C, C], f32, isOutput=False)
    wk_d = nc.declare_dram_parameter("WkT", [C, C], f32, isOutput=False)
    wv_d = nc.declare_dram_parameter("WvT", [C, C], f32, isOutput=False)
    wp_d = nc.declare_dram_parameter("WpT", [C, C], f32, isOutput=False)
    bp_d = nc.declare_dram_parameter("bproj", [1, C], f32, isOutput=False)
    xg_d = nc.declare_dram_parameter("xgT", [C, P2], f32, isOutput=False)
    pf_d = nc.declare_dram_parameter("pflag", [1, P2], f32, isOutput=False)
    attn_d = nc.declare_dram_parameter("attn_rows", [K * 2 * N, N], f32,
                                       isOutput=True)
    rows_d = nc.declare_dram_parameter("rows_out", [2, P, C], f32,
                                       isOutput=True)

    with tile.TileContext(nc) as tc:
        with (
            tc.tile_pool(name="resident", bufs=1) as rp,
            tc.tile_pool(name="tmp", bufs=2) as tp,
            tc.tile_pool(name="stage", bufs=3) as sp,
            tc.tile_pool(name="scr", bufs=4) as cp,
            tc.tile_pool(name="expp", bufs=8) as ep,
            tc.tile_pool(name="psA", bufs=2, space="PSUM") as psA,
            tc.tile_pool(name="psB", bufs=4, space="PSUM") as psB,
        ):
            # ---------------- batched loads + casts ----------------
            # order: cluster-part inputs first (small, unblock PE quickly),
            # then dense inputs, masks last (needed latest)
            xgld = tp.tile([128, 4, P2], f32, name="xgld", tag="xgld")
            nc.sync.dma_start(
                xgld[:], xg_d[:].rearrange("(kc p) n -> p kc n", p=128))
            xg = rp.tile([128, 4, P2], bf16, name="xgb", tag="xgb")
            nc.vector.tensor_copy(xg[:], xgld[:])
            pfld = tp.tile([1, P2], f32, name="pfld", tag="pfld")
            nc.sync.dma_start(pfld[:], pf_d[:])
            pfl = rp.tile([1, P2], bf16, name="pfb", tag="pfb")
            nc.vector.tensor_copy(pfl[:], pfld[:])
            wbf = {}
            for name, dram in (("q", wq_d), ("k", wk_d), ("v", wv_d)):
                ld = tp.tile([128, 4, C], f32, name="wld", tag="wld")
                nc.sync.dma_start(
                    ld[:], dram[:].rearrange("(kc p) n -> p kc n", p=128))
                bt = rp.tile([128, 4, C], bf16, name=f"w{name}b",
                             tag=f"w{name}b")
                if name in ("v", "q"):
                    nc.scalar.copy(bt[:], ld[:])
                else:
                    nc.vector.tensor_copy(bt[:], ld[:])
                wbf[name] = bt
            wh = rp.tile([128, 4, 256], f32, name="wh", tag="wh")
            nc.sync.dma_start(
                wh[:], wh_d[:].rearrange("(kc p) n -> p kc n", p=128))
            xsT = rp.tile([128, 4, N], f32, name="xsTf", tag="xsTf")
            nc.sync.dma_start(
                xsT[:], xsT_d[:].rearrange("(kc p) n -> p kc n", p=128))
            xT = []          # [bb] -> [128, 4, N] f32  (kc in middle dim)
            for bb in range(2):
                t = rp.tile([128, 4, N], f32, name=f"xTf{bb}", tag=f"xTf{bb}")
                nc.sync.dma_start(
                    t[:], xT_d[bb].rearrange("(kc p) n -> p kc n", p=128))
                xT.append(t)
            ridx = rp.tile([128, 16], i32, name="ridx", tag="ridx")
            nc.sync.dma_start(ridx[:], ridx_d[:])
            ldp = tp.tile([128, 4, C], f32, name="wldp", tag="wld")
            nc.sync.dma_start(
                ldp[:], wp_d[:].rearrange("(kc p) n -> p kc n", p=128))
            wpb = rp.tile([128, 4, C], bf16, name="wpb", tag="wpb")
            nc.vector.tensor_copy(wpb[:], ldp[:])
            wbf["p"] = wpb
            msk = []         # [half] -> [128, 4, N] f32 (t = 4*half + mid)
            for half in range(2):
                t = rp.tile([128, 4, N], f32, name=f"msk{half}",
                            tag=f"msk{half}")
                nc.sync.dma_start(
                    t[:], msk_d[4 * 128 * half:4 * 128 * (half + 1),
                                :].rearrange("(tt p) n -> p tt n", p=128))
                msk.append(t)
            if has_bias:
                bld = tp.tile([1, C], f32, name="bld", tag="bld")
                nc.sync.dma_start(bld[:], bp_d[:])
                bpb = rp.tile([1, C], bf16, name="bpb", tag="bpb")
                nc.vector.tensor_copy(bpb[:], bld[:])
            ones_row = rp.tile([1, 128], bf16, name="ones", tag="ones")
            nc.gpsimd.memset(ones_row[:], 1.0)
            ident = rp.tile([128, 128], bf16, name="ident", tag="ident")
            nc.gpsimd.memset(ident[:], 0.0)
            nc.gpsimd.affine_select(
                out=ident[:], in_=ident[:],
                compare_op=mybir.AluOpType.not_equal, fill=1.0, base=0,
                pattern=[[-1, 128]], channel_multiplier=1)

            # ---------------- cluster part: projections first ----------------
            # (small inputs -> PE starts early and HAM-warms during big loads)
            # gathered q/k per head over the 2P concat (+ ext row)
            qg = [None] * H
            kg = [None] * H
            for hp in range(4):
                qp2 = psB.tile([128, P2], f32, name="qp2", tag="psc")
                for kc in range(4):
                    nc.tensor.matmul(
                        qp2[:], wbf["q"][:, kc, 128 * hp:128 * (hp + 1)],
                        xg[:, kc, :], start=(kc == 0), stop=(kc == 3))
                kp2 = psB.tile([128, P2], f32, name="kp2", tag="psc")
                for kc in range(4):
                    nc.tensor.matmul(
                        kp2[:], wbf["k"][:, kc, 128 * hp:128 * (hp + 1)],
                        xg[:, kc, :], start=(kc == 0), stop=(kc == 3))
                for hh in range(2):
                    h = 2 * hp + hh
                    qe = rp.tile([65, P2], bf16, name=f"qg{h}", tag=f"qg{h}")
                    nc.scalar.mul(qe[0:64, :],
                                  qp2[64 * hh:64 * (hh + 1), :], SCALE)
                    nc.gpsimd.memset(qe[64:65, :], 1.0)
                    qg[h] = qe
                    ke = rp.tile([65, P2], bf16, name=f"kg{h}", tag=f"kg{h}")
                    nc.scalar.copy(ke[0:64, :],
                                   kp2[64 * hh:64 * (hh + 1), :])
                    nc.vector.tensor_copy(ke[64:65, :], pfl[:])
                    kg[h] = ke
            # v for all heads at once; per head-pair tiles hold
            # [v_h0 (64) | ones | v_h1 (64) | ones] so the num matmul also
            # produces the softmax denominator Z in its last column.
            vg = [[[None] * 4 for _ in CH] for _ in range(2)]
            for bb in range(2):
                for ci, (off, sz) in enumerate(CH):
                    vp2 = psB.tile([128, C], f32, name="vp2", tag="psc")
                    for kc in range(4):
                        nc.tensor.matmul(
                            vp2[0:sz, :],
                            xg[:, kc, bb * P + off:bb * P + off + sz],
                            wbf["v"][:, kc, :],
                            start=(kc == 0), stop=(kc == 3))
                    for hp in range(4):
                        vt2 = rp.tile([128, 130], bf16,
                                      name=f"vg{bb}{ci}{hp}",
                                      tag=f"vg{bb}{ci}{hp}")
                        nc.scalar.copy(vt2[0:sz, 0:64],
                                       vp2[0:sz, 128 * hp:128 * hp + 64])
                        nc.scalar.copy(vt2[0:sz, 65:129],
                                       vp2[0:sz, 128 * hp + 64:128 * hp + 128])
                        nc.gpsimd.memset(vt2[:, 64:65], 1.0)
                        nc.gpsimd.memset(vt2[:, 129:130], 1.0)
                        vg[bb][ci][hp] = vt2
            # xsum -> Vsum per batch (scaled by eps/N)
            vs = []
            for bb in range(2):
                xsum = [None] * 4
                for kc in range(4):
                    red = cp.tile([128, 1], f32, name="xsum", tag="xsum")
                    nc.vector.reduce_sum(red[:], xT[bb][:, kc, :],
                                         axis=mybir.AxisListType.X)
                    xb = cp.tile([128, 1], bf16, name="xsumb", tag="xsumb")
                    nc.vector.tensor_copy(xb[:], red[:])
                    xsum[kc] = xb
                vp = psB.tile([1, C], f32, name="vp", tag="psc")
                for kc in range(4):
                    nc.tensor.matmul(vp[:], xsum[kc][:], wbf["v"][:, kc, :],
                                     start=(kc == 0), stop=(kc == 3))
                vt = rp.tile([1, C], bf16, name=f"vsum{bb}", tag=f"vsum{bb}")
                nc.scalar.mul(vt[:], vp[:], EPS / N)
                vs.append(vt)

            # ---------------- dense attn_map part ----------------
            # q-sorted / k for both local heads in one [128, N] tile each
            qp = psA.tile([128, N], f32, name="qp", tag="bigs")
            for nb in range(2):
                for kc in range(4):
                    nc.tensor.matmul(
                        qp[:, 512 * nb:512 * (nb + 1)],
                        wh[:, kc, 0:128],
                        xsT[:, kc, 512 * nb:512 * (nb + 1)],
                        start=(kc == 0), stop=(kc == 3))
            qsT = rp.tile([128, N], bf16, name="qsT", tag="qsT")
            nc.scalar.mul(qsT[:], qp[:], SCALE)  # fold attention scale
            kp = psA.tile([128, N], f32, name="kp", tag="bigs")
            for nb in range(2):
                for kc in range(4):
                    nc.tensor.matmul(
                        kp[:, 512 * nb:512 * (nb + 1)],
                        wh[:, kc, 128:256],
                        xT[0][:, kc, 512 * nb:512 * (nb + 1)],
                        start=(kc == 0), stop=(kc == 3))
            kT = rp.tile([128, N], bf16, name="kTt", tag="kTt")
            nc.scalar.copy(kT[:], kp[:])
            # sorted-row score tiles -> mask -> scatter to (c, h, orig) rows
            for hh in range(2):
                for t in range(8):
                    sps = psA.tile([128, N], f32, name="sps", tag="bigs")
                    for nb in range(2):
                        nc.tensor.matmul(
                            sps[:, 512 * nb:512 * (nb + 1)],
                            qsT[64 * hh:64 * (hh + 1), 128 * t:128 * (t + 1)],
                            kT[64 * hh:64 * (hh + 1),
                               512 * nb:512 * (nb + 1)],
                            start=True, stop=True)
                    masked = sp.tile([128, N], f32, name="masked",
                                     tag="masked")
                    nc.vector.tensor_tensor(masked[:], sps[:],
                                            msk[t // 4][:, t % 4, :],
                                            op=mybir.AluOpType.mult)
                    nc.gpsimd.indirect_dma_start(
                        out=attn_d[:],
                        out_offset=bass.IndirectOffsetOnAxis(
                            ap=ridx[:, 8 * hh + t:8 * hh + t + 1], axis=0),
                        in_=masked[:],
                        in_offset=None)

            # ------------- cluster attention (eps-smoothed) -------------
            obf = [[rp.tile([128, C], bf16, name=f"obf{bb}{ci}",
                            tag=f"obf{bb}{ci}") for ci in range(len(CH))]
                   for bb in range(2)]
            for bb in range(2):
                for h in range(H):
                    hp, hh = divmod(h, 2)
                    # key-major exp tiles
                    expT = []
                    for (joff, jsz) in CH:
                        spT = psB.tile([128, P], f32, name="spT", tag="psc")
                        nc.tensor.matmul(
                            spT[0:jsz, :],
                            kg[h][:, bb * P + joff:bb * P + joff + jsz],
                            qg[h][:, bb * P:bb * P + P],
                            start=True, stop=True)
                        et = ep.tile([128, P], bf16, name="expT", tag="expT")
                        nc.scalar.activation(et[0:jsz, :], spT[0:jsz, :],
                                             AF.Exp)
                        expT.append(et)
                    # num[:, 0:64] = exp @ v + (eps/N) * Vsum,
                    # num[:, 64] = Z; rows scaled by 1/(Z+eps)
                    for ci, (ioff, isz) in enumerate(CH):
                        np_ = psB.tile([128, 65], f32, name="nump", tag="psc")
                        for ji, (joff, jsz) in enumerate(CH):
                            nc.tensor.matmul(
                                np_[0:isz, :],
                                expT[ji][0:jsz, ioff:ioff + isz],
                                vg[bb][ji][hp][0:jsz,
                                               65 * hh:65 * hh + 65],
                                start=(ji == 0), stop=False)
                        nc.tensor.matmul(np_[0:isz, 0:64],
                                         ones_row[:, 0:isz],
                                         vs[bb][:, 64 * h:64 * (h + 1)],
                                         start=False, stop=True,
                                         skip_group_check=True)
                        ze = cp.tile([128, 1], f32, name="ze", tag="ze")
                        nc.vector.tensor_scalar_add(ze[0:isz, :],
                                                    np_[0:isz, 64:65], EPS)
                        rc = cp.tile([128, 1], f32, name="rc", tag="rc",
                                     bufs=6)
                        nc.vector.reciprocal(rc[0:isz, :], ze[0:isz, :])
                        nc.vector.tensor_scalar_mul(
                            obf[bb][ci][0:isz, 64 * h:64 * (h + 1)],
                            np_[0:isz, 0:64], rc[0:isz, :])

            # transpose o, project, store out rows
            for bb in range(2):
                oT = [[None] * len(CH) for _ in range(4)]
                for ci, (ioff, isz) in enumerate(CH):
                    for cc in range(4):
                        tps = psB.tile([128, 128], bf16, name="psc_t",
                                       tag="psc")
                        nc.tensor.transpose(
                            tps[:, 0:isz],
                            obf[bb][ci][0:isz, 128 * cc:128 * (cc + 1)],
                            ident[0:isz, 0:isz])
                        ot = cp.tile([128, 128], bf16, name=f"oT{bb}{cc}{ci}",
                                     tag=f"oT{cc}{ci}")
                        nc.scalar.copy(ot[:, 0:isz], tps[:, 0:isz])
                        oT[cc][ci] = ot
                for ci, (ioff, isz) in enumerate(CH):
                    fp = psB.tile([128, C], f32, name="fp", tag="psc")
                    for cc in range(4):
                        nc.tensor.matmul(fp[0:isz, :], oT[cc][ci][:, 0:isz],
                                         wbf["p"][:, cc, :],
                                         start=(cc == 0),
                                         stop=(cc == 3 and not has_bias))
                    if has_bias:
                        nc.tensor.matmul(fp[0:isz, :], ones_row[:, 0:isz],
                                         bpb[:], start=False, stop=True)
                    fs = sp.tile([128, C], f32, name="fs", tag="fs")
                    nc.scalar.copy(fs[0:isz, :], fp[0:isz, :])
                    nc.sync.dma_start(rows_d[bb, ioff:ioff + isz, :],
                                      fs[0:isz, :])

    nc.compile()
    return nc


def kernel(**inputs):
    from concourse.bass_utils import run_bass_kernel_spmd

    x = np.asarray(inputs["x_token"], np.float32)             # (B, N, C)
    idx = np.asarray(inputs["idx_cluster"]).astype(np.int64)  # (B, N)
    Wq = np.asarray(inputs["Wq"], np.float32)
    Wk = np.asarray(inputs["Wk"], np.float32)
    Wv = np.asarray(inputs["Wv"], np.float32)
    Wproj = np.asarray(inputs["Wproj"], np.float32)
    bproj = np.asarray(inputs["bproj"], np.float32)
    assert x.shape == (B, N, C) and idx.shape == (B, N)
    assert int(np.asarray(inputs["cluster_num"])) == K

    # ---- host-side index/shard prep
    perm = [np.argsort(idx[b], kind="stable") for b in range(B)]
    sortc = [idx[b][perm[b]] for b in range(B)]
    ids = [[np.where(idx[b] == c)[0] for c in range(K)] for b in range(B)]
    maxsz = max(len(ids[b][c]) for b in range(B) for c in range(K))
    P = max(32, 32 * math.ceil(maxsz / 32))

    xT = [np.ascontiguousarray(x[b].T) for b in range(B)]
    xsT = [np.ascontiguousarray(x[b][perm[b]].T) for b in range(B)]
    msk = [(sortc[b][:, None] == idx[b][None, :]).astype(np.float32)
           for b in range(B)]
    WqT = np.ascontiguousarray(Wq.T)
    WkT = np.ascontiguousarray(Wk.T)
    WvT = np.ascontiguousarray(Wv.T)
    WpT = np.ascontiguousarray(Wproj.T)
    has_bias = bool(np.any(bproj != 0))

    in_maps = []
    for core in range(NCORES):
        b = core // 4
        h0 = 2 * (core % 4)
        c = core
        # scatter row ids: shard row = (cluster*2 + h_local)*N + orig_row
        ridx = np.zeros((128, 16), np.int32)
        for hl in range(2):
            rows = (sortc[b] * 2 + hl) * N + perm[b]
            for t in range(8):
                ridx[:, 8 * hl + t] = rows[128 * t:128 * (t + 1)]
        # gathered cluster tokens, [own batch | other batch] concat
        xgT = np.zeros((C, 2 * P), np.float32)
        pfl = np.zeros((1, 2 * P), np.float32)
        for slot, bb in enumerate((b, 1 - b)):
            tok = ids[bb][c]
            xgT[:, slot * P:slot * P + len(tok)] = x[bb][tok].T
            pfl[0, slot * P + len(tok):(slot + 1) * P] = -1e9
        in_maps.append({
            "xT": np.ascontiguousarray(np.stack([xT[b], xT[1 - b]])),
            "xsT": xsT[b],
            "msk": msk[b],
            "ridx": ridx,
            "Wh": np.ascontiguousarray(np.concatenate(
                [WqT[:, 64 * h0:64 * (h0 + 2)],
                 WkT[:, 64 * h0:64 * (h0 + 2)]], axis=1)),
            "WqT": WqT, "WkT": WkT, "WvT": WvT, "WpT": WpT,
            "bproj": np.ascontiguousarray(bproj.reshape(1, C)),
            "xgT": xgT,
            "pflag": pfl,
        })

    nc = _build(P, has_bias)
    trace = bool(os.environ.get("CTA_TRACE"))
    res = run_bass_kernel_spmd(nc, in_maps, core_ids=list(range(NCORES)),
                               trace=trace)
    global LAST_EXEC_NS, LAST_RES
    LAST_EXEC_NS = res.exec_time_ns
    LAST_RES = res

    # ---- unshard
    attn_map = np.empty((B, K, H, N, N), np.float32)
    out = np.empty((B, N, C), np.float32)
    for core in range(NCORES):
        b = core // 4
        h0 = 2 * (core % 4)
        c = core
        shard = res.results[core]["attn_rows"].reshape(K, 2, N, N)
        attn_map[b, :, h0:h0 + 2] = shard
        rows = res.results[core]["rows_out"]          # (2, P, C)
        for slot, bb in enumerate((b, 1 - b)):
            tok = ids[bb][c]
            out[bb, tok] = rows[slot, :len(tok)]
    return out, attn_map
